# revision 20
# baseline (speedup 1.0000x reference)
"""Trainium2 Bass kernel for expected-calibration-error (ECE) over [N,C] logits.

Contract: kernel(logits, targets) -> np.float32 scalar (shape ()), matching

    probs = softmax(logits); conf = max(probs); pred = argmax(probs)
    acc = (pred == targets); bins of width 1/10 over (k/10, (k+1)/10]
    ECE = sum_k |avg_conf_k - avg_acc_k| * count_k / N

Strategy (data-parallel over 8 NeuronCores, rows sharded, bf16 on device):
  * Host casts logits to bf16 (halves HBM traffic; 2.6e-3 final rel err vs
    the 2e-2 gate) and gathers tval[i] = y16[i, targets[i]]. Since tval is
    an element of row i, acc == (tval == rowmax) reproduces argmax accuracy
    up to bf16 ties.
  * Rows live on partitions; each row's 128 classes are stored in host
    bit-reversed block order [b6,b5,b4,b3,t,c_low3] so every pairwise fold
    level of a reduction tree combines two CONTIGUOUS halves. DVE
    tensor_tensor measures 2 elem/cycle/lane on bf16 (fp16 add is
    emulated ~6x slower; tensor_reduce is 1 elem/cycle) — so per-row max
    and sum-of-exp run as contiguous bf16 folds + one batched width-8
    reduce per group.
  * exp on the ACT engine (bf16 in/out; an f32 E was measured to slow
    concurrent DVE via SBUF bandwidth). GpSimd stays fully idle: any
    sustained Pool-engine work starves DVE SBUF access 3-20x. Both
    reduction trees therefore run on DVE; width-8 tails are reduced once
    per group (batched tensor_reduce) to cut per-instruction overheads.
    Tile 0 loads/computes in two halves to shorten pipeline fill.
  * conf = exp(max) * recip(sumexp); cumulative bin masks g[k] = conf>k/10
    as [P, 11, gw] bf16; per-128-row-block matmul(lhsT=[1|conf|acc], rhs=g)
    accumulates cumulative [3,11] stats in PSUM across the whole shard.
  * Host sums the 8 [3,11] outputs, differences adjacent cumulative
    columns, applies the final ECE formula.
"""

import numpy as np

# Problem constants (hardcoded per harness contract).
N = 262144
C = 128
P = 128          # SBUF partitions
NB = 10          # calibration bins
NCORES = 8
T = 32           # rows per partition per tile
GK = 4           # tiles per small-op group
ROWS_PER_CORE = N // NCORES          # 32768
NTILES = ROWS_PER_CORE // (P * T)    # 8
NGROUPS = NTILES // GK
GW = GK * T                          # group width (stat columns)
FREE = T * C                         # 4096 elems per partition per tile

_CACHE = {}

KNOBS = dict(gk=2, io_bufs=4, e_bufs=3, f_bufs=3)


def build(gk=2, io_bufs=4, e_bufs=3, f_bufs=3):
    import concourse.bacc as bacc
    import concourse.tile as tile
    from concourse import mybir

    f16 = mybir.dt.float16
    bf16 = mybir.dt.bfloat16
    f32 = mybir.dt.float32
    Alu = mybir.AluOpType
    Act = mybir.ActivationFunctionType
    X = mybir.AxisListType.X

    ngroups = NTILES // gk
    gw = gk * T

    nc = bacc.Bacc(trn_type="TRN2")

    y_d = nc.dram_tensor("y", [NTILES, P, FREE], bf16, kind="ExternalInput")
    tv_d = nc.dram_tensor("tv", [ngroups, P, gw], bf16, kind="ExternalInput")
    thr_d = nc.dram_tensor("thr", [1, (NB + 1) * gw], bf16, kind="ExternalInput")
    out_d = nc.dram_tensor("gstats", [3, NB + 1], f32, kind="ExternalOutput")

    with nc.allow_low_precision("bf16 ECE pipeline; 2.6e-3 final rel err"):
        with tile.TileContext(nc) as tc:
            with (
                tc.tile_pool(name="io", bufs=io_bufs) as io_pool,
                tc.tile_pool(name="eb", bufs=e_bufs) as eb_pool,
                tc.tile_pool(name="fm", bufs=f_bufs) as fm_pool,
                tc.tile_pool(name="fs", bufs=f_bufs) as fs_pool,
                tc.tile_pool(name="grp", bufs=3) as grp_pool,
                tc.tile_pool(name="single", bufs=1) as single,
                tc.tile_pool(name="psum", bufs=1, space="PSUM") as psum_pool,
            ):
                # tile 0 loads as two halves so DVE/ACT start ~2us earlier;
                # each half of the bit-reversed layout covers half the classes
                y_first = io_pool.tile([P, FREE], bf16, name="y0")
                nc.sync.dma_start(out=y_first[:, 0:FREE // 2], in_=y_d[0][:, 0:FREE // 2])
                nc.sync.dma_start(out=y_first[:, FREE // 2:FREE], in_=y_d[0][:, FREE // 2:FREE])

                thrg = single.tile([P, (NB + 1) * gw], bf16)
                thrg3 = thrg[:].rearrange("p (a b) -> p a b", b=gw)

                pstats = psum_pool.tile([3, NB + 1], f32)

                def fold(src, out8, op, pool, tag, half=False):
                    """src [P, 4096] (bit-reversed) -> out8 [P, T*8]: contiguous
                    pairwise folds down to width 8 per row; the final width-8
                    reduce happens once per group (batched) to cut overheads."""
                    cur, w = src, (FREE // 2 if half else FREE)
                    while w > 2 * T * 8:
                        h = w // 2
                        nxt = pool.tile([P, h], bf16, name=f"{tag}{h}")
                        nc.vector.tensor_tensor(
                            out=nxt[:], in0=cur[:, 0:h], in1=cur[:, h:w], op=op
                        )
                        cur, w = nxt[:], h
                    h = w // 2
                    nc.vector.tensor_tensor(
                        out=out8, in0=cur[:, 0:h], in1=cur[:, h:w], op=op
                    )

                for grp in range(ngroups):
                    my_g = grp_pool.tile([P, gw], bf16)
                    s_g = grp_pool.tile([P, gw], bf16)
                    g8m = grp_pool.tile([P, gw * 8], bf16)
                    g8s = grp_pool.tile([P, gw * 8], bf16)
                    tv_g = grp_pool.tile([P, gw], bf16)
                    nc.sync.dma_start(out=tv_g[:], in_=tv_d[grp])

                    for ti in range(gk):
                        t = grp * gk + ti
                        o0, o1 = ti * T, (ti + 1) * T

                        m8 = g8m[:, o0 * 8 : o1 * 8]
                        s8 = g8s[:, o0 * 8 : o1 * 8]
                        if t == 0:
                            # halved pipeline: per-half folds + tiny combine
                            y_t = y_first
                            H = FREE // 2
                            mh = grp_pool.tile([P, 2, T * 8], bf16, name="mh")
                            sh = grp_pool.tile([P, 2, T * 8], bf16, name="sh")
                            E = eb_pool.tile([P, FREE], bf16)
                            for hf in range(2):
                                yh = y_t[:, hf * H : (hf + 1) * H]
                                fold(yh, mh[:, hf, :], Alu.max, fm_pool, "m",
                                     half=True)
                                Eh = E[:, hf * H : (hf + 1) * H]
                                nc.scalar.activation(out=Eh, in_=yh, func=Act.Exp)
                                fold(Eh, sh[:, hf, :], Alu.add, fs_pool, "s",
                                     half=True)
                            nc.vector.tensor_tensor(
                                out=m8, in0=mh[:, 0, :], in1=mh[:, 1, :], op=Alu.max
                            )
                            nc.vector.tensor_tensor(
                                out=s8, in0=sh[:, 0, :], in1=sh[:, 1, :], op=Alu.add
                            )
                        else:
                            y_t = io_pool.tile([P, FREE], bf16)
                            nc.sync.dma_start(out=y_t[:], in_=y_d[t])

                            # per-row max of y (DVE folds; Pool has no max op)
                            fold(y_t[:], m8, Alu.max, fm_pool, "m")

                            # per-row sum of exp(y); bf16 E (f32 E doubles ACT's
                            # SBUF write traffic and slows concurrent DVE)
                            E = eb_pool.tile([P, FREE], bf16)
                            nc.scalar.activation(out=E[:], in_=y_t[:], func=Act.Exp)
                            fold(E[:], s8, Alu.add, fs_pool, "s")

                    if grp == 0:
                        # thresholds aren't needed until phase 2; issuing the
                        # broadcast here keeps it from delaying tile loads
                        nc.sync.dma_start(
                            out=thrg[:], in_=thr_d[:].partition_broadcast(P)
                        )

                    # batched width-8 reduces for the whole group
                    nc.vector.tensor_reduce(
                        out=my_g[:], in_=g8m[:].rearrange("p (t c) -> p t c", c=8),
                        axis=X, op=Alu.max,
                    )
                    nc.vector.tensor_reduce(
                        out=s_g[:], in_=g8s[:].rearrange("p (t c) -> p t c", c=8),
                        axis=X, op=Alu.add,
                    )

                    # --- batched small per-row ops over the whole group ---
                    maxE = grp_pool.tile([P, gw], bf16)
                    nc.scalar.activation(out=maxE[:], in_=my_g[:], func=Act.Exp)
                    rs = grp_pool.tile([P, gw], bf16)
                    nc.vector.reciprocal(out=rs[:], in_=s_g[:])

                    rhs3 = grp_pool.tile([P, 3, gw], bf16)
                    nc.vector.memset(rhs3[:, 0, :], 1.0)
                    nc.vector.tensor_tensor(
                        out=rhs3[:, 1, :], in0=maxE[:], in1=rs[:], op=Alu.mult
                    )
                    # tval is an element of the row, so tval == max <=> argmax hit
                    nc.vector.tensor_tensor(
                        out=rhs3[:, 2, :], in0=tv_g[:], in1=my_g[:], op=Alu.is_equal
                    )

                    g = grp_pool.tile([P, NB + 1, gw], bf16)
                    half = gw // 2
                    for hf in range(2):
                        s0, s1 = hf * half, (hf + 1) * half
                        conf_b = rhs3[:, 1, s0:s1].unsqueeze(1).broadcast_to(
                            [P, NB + 1, half]
                        )
                        nc.vector.tensor_tensor(
                            out=g[:, :, s0:s1], in0=conf_b,
                            in1=thrg3[:, :, s0:s1], op=Alu.is_gt,
                        )
                        for j in range(s0, s1):
                            nc.tensor.matmul(
                                pstats[:],
                                rhs3[:, :, j],
                                g[:, :, j],
                                start=(grp == 0 and j == 0),
                                stop=(grp == ngroups - 1 and j == gw - 1),
                                skip_group_check=True,
                            )

                stats_sb = single.tile([3, NB + 1], f32)
                nc.scalar.copy(out=stats_sb[:], in_=pstats[:])
                nc.sync.dma_start(out=out_d[:], in_=stats_sb[:])

    nc.compile()
    return nc


def prep_inputs(logits, targets):
    """Cast + shard + fold-layout host inputs. Returns per-core in_maps."""
    import ml_dtypes

    bf16 = ml_dtypes.bfloat16
    l = np.asarray(logits, dtype=np.float32)
    tg = np.asarray(targets).astype(np.int64)
    n = l.shape[0]

    y16 = l.astype(bf16)
    tval = y16[np.arange(n), tg]

    thrv = (np.arange(NB + 1, dtype=np.float32) / NB).astype(bf16)
    thrv[NB] = bf16(3e38)  # bin mask 10 must stay empty even if conf rounds >1
    thr_flat = np.ascontiguousarray(
        np.repeat(thrv, KNOBS["gk"] * T).reshape(1, (NB + 1) * KNOBS["gk"] * T)
    )

    gk = KNOBS["gk"]
    ngroups = NTILES // gk
    gw = gk * T
    rpc = n // NCORES
    in_maps = []
    for k in range(NCORES):
        yk = y16[k * rpc : (k + 1) * rpc].reshape(NTILES, P, T, C)
        # bit-reversed block layout: [t, c=(b6 b5 b4 b3 c3)] -> [b6 b5 b4 b3 t c3]
        yk = (
            yk.reshape(NTILES, P, T, 2, 2, 2, 2, 8)
            .transpose(0, 1, 3, 4, 5, 6, 2, 7)
            .reshape(NTILES, P, FREE)
        )
        tvk = (
            tval[k * rpc : (k + 1) * rpc]
            .reshape(ngroups, gk, P, T)
            .transpose(0, 2, 1, 3)
            .reshape(ngroups, P, gw)
        )
        in_maps.append(
            {
                "y": np.ascontiguousarray(yk),
                "tv": np.ascontiguousarray(tvk),
                "thr": thr_flat,
            }
        )
    return in_maps


def finalize(gstats_list, n=N):
    """Combine per-core cumulative [3, 11] stats into the ECE scalar."""
    G = np.zeros((3, NB + 1), dtype=np.float64)
    for gs in gstats_list:
        G += gs.astype(np.float64)
    per = G[:, 0:NB] - G[:, 1 : NB + 1]
    counts, sum_conf, sum_acc = per[0], per[1], per[2]
    safe = np.maximum(counts, 1.0)
    avg_conf = sum_conf / safe
    avg_acc = sum_acc / safe
    prop = counts / float(n)
    ece = np.where(counts > 0, np.abs(avg_conf - avg_acc) * prop, 0.0).sum()
    return np.array(ece, dtype=np.float32)


LAST_RESULTS = None  # BassKernelResults of the most recent kernel() call


def kernel(logits, targets):
    global LAST_RESULTS
    from concourse.bass_utils import run_bass_kernel_spmd

    key = tuple(sorted(KNOBS.items()))
    if key not in _CACHE:
        _CACHE[key] = build(**KNOBS)
    nc = _CACHE[key]

    in_maps = prep_inputs(logits, targets)
    res = run_bass_kernel_spmd(nc, in_maps, core_ids=list(range(NCORES)))
    LAST_RESULTS = res
    return finalize([r["gstats"] for r in res.results])


# revision 21
# speedup vs baseline: 1.1791x; 1.1791x over previous
"""Trainium2 Bass kernel for expected-calibration-error (ECE) over [N,C] logits.

Contract: kernel(logits, targets) -> np.float32 scalar (shape ()), matching

    probs = softmax(logits); conf = max(probs); pred = argmax(probs)
    acc = (pred == targets); bins of width 1/10 over (k/10, (k+1)/10]
    ECE = sum_k |avg_conf_k - avg_acc_k| * count_k / N

Strategy (data-parallel over 8 NeuronCores, rows sharded, bf16 on device):
  * Host casts logits to bf16 (halves HBM traffic; 2.6e-3 final rel err vs
    the 2e-2 gate) and gathers tval[i] = y16[i, targets[i]]. Since tval is
    an element of row i, acc == (tval == rowmax) reproduces argmax accuracy
    up to bf16 ties.
  * Rows live on partitions; each row's 128 classes are stored in host
    bit-reversed block order [b6,b5,b4,b3,t,c_low3] so every pairwise fold
    level of a reduction tree combines two CONTIGUOUS halves. DVE
    tensor_tensor measures 2 elem/cycle/lane on bf16 (fp16 add is
    emulated ~6x slower; tensor_reduce is 1 elem/cycle) — so per-row max
    and sum-of-exp run as contiguous bf16 folds + one batched width-8
    reduce per group.
  * exp on the ACT engine (bf16 in/out; an f32 E was measured to slow
    concurrent DVE via SBUF bandwidth). GpSimd stays fully idle: any
    sustained Pool-engine work starves DVE SBUF access 3-20x. Both
    reduction trees therefore run on DVE; width-8 tails are reduced once
    per group (batched tensor_reduce) to cut per-instruction overheads.
    Tile 0 loads/computes in two halves to shorten pipeline fill.
  * conf = exp(max) * recip(sumexp); cumulative bin masks g[k] = conf>k/10
    as [P, 11, gw] bf16; per-128-row-block matmul(lhsT=[1|conf|acc], rhs=g)
    accumulates cumulative [3,11] stats in PSUM across the whole shard.
  * Host sums the 8 [3,11] outputs, differences adjacent cumulative
    columns, applies the final ECE formula.
"""

import numpy as np

# Problem constants (hardcoded per harness contract).
N = 262144
C = 128
P = 128          # SBUF partitions
NB = 10          # calibration bins
NCORES = 8
T = 32           # rows per partition per tile
GK = 4           # tiles per small-op group
ROWS_PER_CORE = N // NCORES          # 32768
NTILES = ROWS_PER_CORE // (P * T)    # 8
NGROUPS = NTILES // GK
GW = GK * T                          # group width (stat columns)
FREE = T * C                         # 4096 elems per partition per tile

_CACHE = {}

KNOBS = dict(gk=2, io_bufs=4, e_bufs=3, f_bufs=3)


def build(gk=2, io_bufs=4, e_bufs=3, f_bufs=3):
    import concourse.bacc as bacc
    import concourse.tile as tile
    from concourse import mybir

    f16 = mybir.dt.float16
    bf16 = mybir.dt.bfloat16
    f32 = mybir.dt.float32
    Alu = mybir.AluOpType
    Act = mybir.ActivationFunctionType
    X = mybir.AxisListType.X

    ngroups = NTILES // gk
    gw = gk * T

    nc = bacc.Bacc(trn_type="TRN2")

    y_d = nc.dram_tensor("y", [NTILES, P, FREE], bf16, kind="ExternalInput")
    tv_d = nc.dram_tensor("tv", [ngroups, P, gw], bf16, kind="ExternalInput")
    thr_d = nc.dram_tensor("thr", [1, (NB + 1) * gw], bf16, kind="ExternalInput")
    out_d = nc.dram_tensor("gstats", [3, NB + 1], f32, kind="ExternalOutput")

    with nc.allow_low_precision("bf16 ECE pipeline; 2.6e-3 final rel err"):
        with tile.TileContext(nc) as tc:
            with (
                tc.tile_pool(name="io", bufs=io_bufs) as io_pool,
                tc.tile_pool(name="eb", bufs=e_bufs) as eb_pool,
                tc.tile_pool(name="fm", bufs=f_bufs) as fm_pool,
                tc.tile_pool(name="fs", bufs=f_bufs) as fs_pool,
                tc.tile_pool(name="grp", bufs=3) as grp_pool,
                tc.tile_pool(name="single", bufs=1) as single,
                tc.tile_pool(name="psum", bufs=1, space="PSUM") as psum_pool,
            ):
                # tile 0 loads as two halves so DVE/ACT start ~2us earlier;
                # each half of the bit-reversed layout covers half the classes
                y_first = io_pool.tile([P, FREE], bf16, name="y0")
                nc.sync.dma_start(out=y_first[:, 0:FREE // 2], in_=y_d[0][:, 0:FREE // 2])
                nc.sync.dma_start(out=y_first[:, FREE // 2:FREE], in_=y_d[0][:, FREE // 2:FREE])

                thrg = single.tile([P, (NB + 1) * gw], bf16)
                nc.sync.dma_start(out=thrg[:], in_=thr_d[:].partition_broadcast(P))
                thrg3 = thrg[:].rearrange("p (a b) -> p a b", b=gw)

                pstats = psum_pool.tile([3, NB + 1], f32)

                def fold(src, out8, op, pool, tag, half=False):
                    """src [P, 4096] (bit-reversed) -> out8 [P, T*8]: contiguous
                    pairwise folds down to width 8 per row; the final width-8
                    reduce happens once per group (batched) to cut overheads."""
                    cur, w = src, (FREE // 2 if half else FREE)
                    while w > 2 * T * 8:
                        h = w // 2
                        nxt = pool.tile([P, h], bf16, name=f"{tag}{h}")
                        nc.vector.tensor_tensor(
                            out=nxt[:], in0=cur[:, 0:h], in1=cur[:, h:w], op=op
                        )
                        cur, w = nxt[:], h
                    h = w // 2
                    nc.vector.tensor_tensor(
                        out=out8, in0=cur[:, 0:h], in1=cur[:, h:w], op=op
                    )

                for grp in range(ngroups):
                    my_g = grp_pool.tile([P, gw], bf16)
                    s_g = grp_pool.tile([P, gw], bf16)
                    g8m = grp_pool.tile([P, gw * 8], bf16)
                    g8s = grp_pool.tile([P, gw * 8], bf16)
                    tv_g = grp_pool.tile([P, gw], bf16)
                    nc.sync.dma_start(out=tv_g[:], in_=tv_d[grp])

                    for ti in range(gk):
                        t = grp * gk + ti
                        o0, o1 = ti * T, (ti + 1) * T

                        m8 = g8m[:, o0 * 8 : o1 * 8]
                        s8 = g8s[:, o0 * 8 : o1 * 8]
                        if t == 0:
                            # halved pipeline: per-half folds + tiny combine
                            y_t = y_first
                            H = FREE // 2
                            mh = grp_pool.tile([P, 2, T * 8], bf16, name="mh")
                            sh = grp_pool.tile([P, 2, T * 8], bf16, name="sh")
                            E = eb_pool.tile([P, FREE], bf16)
                            for hf in range(2):
                                yh = y_t[:, hf * H : (hf + 1) * H]
                                fold(yh, mh[:, hf, :], Alu.max, fm_pool, "m",
                                     half=True)
                                Eh = E[:, hf * H : (hf + 1) * H]
                                nc.scalar.activation(out=Eh, in_=yh, func=Act.Exp)
                                fold(Eh, sh[:, hf, :], Alu.add, fs_pool, "s",
                                     half=True)
                            nc.vector.tensor_tensor(
                                out=m8, in0=mh[:, 0, :], in1=mh[:, 1, :], op=Alu.max
                            )
                            nc.vector.tensor_tensor(
                                out=s8, in0=sh[:, 0, :], in1=sh[:, 1, :], op=Alu.add
                            )
                        else:
                            y_t = io_pool.tile([P, FREE], bf16)
                            nc.sync.dma_start(out=y_t[:], in_=y_d[t])

                            # per-row max of y (DVE folds; Pool has no max op)
                            fold(y_t[:], m8, Alu.max, fm_pool, "m")

                            # per-row sum of exp(y); bf16 E (f32 E doubles ACT's
                            # SBUF write traffic and slows concurrent DVE)
                            E = eb_pool.tile([P, FREE], bf16)
                            nc.scalar.activation(out=E[:], in_=y_t[:], func=Act.Exp)
                            fold(E[:], s8, Alu.add, fs_pool, "s")

                    # batched width-8 reduces for the whole group
                    nc.vector.tensor_reduce(
                        out=my_g[:], in_=g8m[:].rearrange("p (t c) -> p t c", c=8),
                        axis=X, op=Alu.max,
                    )
                    nc.vector.tensor_reduce(
                        out=s_g[:], in_=g8s[:].rearrange("p (t c) -> p t c", c=8),
                        axis=X, op=Alu.add,
                    )

                    # --- batched small per-row ops over the whole group ---
                    maxE = grp_pool.tile([P, gw], bf16)
                    nc.scalar.activation(out=maxE[:], in_=my_g[:], func=Act.Exp)
                    rs = grp_pool.tile([P, gw], bf16)
                    nc.vector.reciprocal(out=rs[:], in_=s_g[:])

                    rhs3 = grp_pool.tile([P, 3, gw], bf16)
                    nc.vector.memset(rhs3[:, 0, :], 1.0)
                    nc.vector.tensor_tensor(
                        out=rhs3[:, 1, :], in0=maxE[:], in1=rs[:], op=Alu.mult
                    )
                    # tval is an element of the row, so tval == max <=> argmax hit
                    nc.vector.tensor_tensor(
                        out=rhs3[:, 2, :], in0=tv_g[:], in1=my_g[:], op=Alu.is_equal
                    )

                    g = grp_pool.tile([P, NB + 1, gw], bf16)
                    conf_b = rhs3[:, 1, :].unsqueeze(1).broadcast_to([P, NB + 1, gw])
                    nc.vector.tensor_tensor(
                        out=g[:], in0=conf_b, in1=thrg3, op=Alu.is_gt
                    )

                    for j in range(gw):
                        nc.tensor.matmul(
                            pstats[:],
                            rhs3[:, :, j],
                            g[:, :, j],
                            start=(grp == 0 and j == 0),
                            stop=(grp == ngroups - 1 and j == gw - 1),
                            skip_group_check=True,
                        )

                stats_sb = single.tile([3, NB + 1], f32)
                nc.scalar.copy(out=stats_sb[:], in_=pstats[:])
                nc.sync.dma_start(out=out_d[:], in_=stats_sb[:])

    nc.compile()
    return nc


def prep_inputs(logits, targets):
    """Cast + shard + fold-layout host inputs. Returns per-core in_maps."""
    import ml_dtypes

    bf16 = ml_dtypes.bfloat16
    l = np.asarray(logits, dtype=np.float32)
    tg = np.asarray(targets).astype(np.int64)
    n = l.shape[0]

    y16 = l.astype(bf16)
    tval = y16[np.arange(n), tg]

    thrv = (np.arange(NB + 1, dtype=np.float32) / NB).astype(bf16)
    thrv[NB] = bf16(3e38)  # bin mask 10 must stay empty even if conf rounds >1
    thr_flat = np.ascontiguousarray(
        np.repeat(thrv, KNOBS["gk"] * T).reshape(1, (NB + 1) * KNOBS["gk"] * T)
    )

    gk = KNOBS["gk"]
    ngroups = NTILES // gk
    gw = gk * T
    rpc = n // NCORES
    in_maps = []
    for k in range(NCORES):
        yk = y16[k * rpc : (k + 1) * rpc].reshape(NTILES, P, T, C)
        # bit-reversed block layout: [t, c=(b6 b5 b4 b3 c3)] -> [b6 b5 b4 b3 t c3]
        yk = (
            yk.reshape(NTILES, P, T, 2, 2, 2, 2, 8)
            .transpose(0, 1, 3, 4, 5, 6, 2, 7)
            .reshape(NTILES, P, FREE)
        )
        tvk = (
            tval[k * rpc : (k + 1) * rpc]
            .reshape(ngroups, gk, P, T)
            .transpose(0, 2, 1, 3)
            .reshape(ngroups, P, gw)
        )
        in_maps.append(
            {
                "y": np.ascontiguousarray(yk),
                "tv": np.ascontiguousarray(tvk),
                "thr": thr_flat,
            }
        )
    return in_maps


def finalize(gstats_list, n=N):
    """Combine per-core cumulative [3, 11] stats into the ECE scalar."""
    G = np.zeros((3, NB + 1), dtype=np.float64)
    for gs in gstats_list:
        G += gs.astype(np.float64)
    per = G[:, 0:NB] - G[:, 1 : NB + 1]
    counts, sum_conf, sum_acc = per[0], per[1], per[2]
    safe = np.maximum(counts, 1.0)
    avg_conf = sum_conf / safe
    avg_acc = sum_acc / safe
    prop = counts / float(n)
    ece = np.where(counts > 0, np.abs(avg_conf - avg_acc) * prop, 0.0).sum()
    return np.array(ece, dtype=np.float32)


LAST_RESULTS = None  # BassKernelResults of the most recent kernel() call


def kernel(logits, targets):
    global LAST_RESULTS
    from concourse.bass_utils import run_bass_kernel_spmd

    key = tuple(sorted(KNOBS.items()))
    if key not in _CACHE:
        _CACHE[key] = build(**KNOBS)
    nc = _CACHE[key]

    in_maps = prep_inputs(logits, targets)
    res = run_bass_kernel_spmd(nc, in_maps, core_ids=list(range(NCORES)))
    LAST_RESULTS = res
    return finalize([r["gstats"] for r in res.results])
